# revision 48
# baseline (speedup 1.0000x reference)
"""Attention-LSTM decoder (LAS-style) Trainium2 Bass kernel.

Sharding: data-parallel over batch N=64 -> 8 cores x 8 examples.

Device strategy (per core, b=8 examples):
 - Recurrent matmuls are batch-stationary (lhsT = state columns [K, 8]) with
   weights streamed as float32r (full-rate fp32 for N>=256).
 - emb(x_t) @ W_ih1[:, :512].T is precomputed for all steps in phase A
   (indirect-DMA gather + big matmul), biases folded in.
 - sigmoid via tanh identity; states stored scaled by 2 (S=2c, H=2h) so each
   gate is one scalar_tensor_tensor; the 2x on h is compensated by
   pre-halving W_hh1/W_ih2/W_hh2/keys/W_out[:, :128] on the host.
 - Attention energy computed DENSE [8, 500] by accumulating 8 matmuls whose
   stationary is H2t masked to one column (block-diag trick); pad mask added
   via an I8 matmul; exp uses accum_out for the softmax denominator (no max
   subtraction -- energies are small, verified against the reference).
 - ctx computed per (example, T-chunk) with stationary val chunks; lands
   transposed [128, 8] = exactly the layout the next-step z1 matmul and the
   final output matmul need.
 - Phase C: [1600, 256] @ [256, 8000] from the stored h2/ctx histories.
"""

import os
from contextlib import ExitStack

import numpy as np

V, H, KS, VS, T, N, L = 8000, 512, 128, 128, 500, 64, 200
NCORES = 8
B = N // NCORES          # 8 examples per core
TCH = 4                  # T chunks for ctx matmuls
TSUB = T // TCH          # 125
HCH = H // 128           # 4 chunks of the h1 dim
G1 = 4 * H               # 2048
G2 = 4 * KS              # 512
NEG = -1e9

_cache = {}


def _nt_mch(nsteps):
    nt = B * nsteps
    return nt, (nt + 127) // 128


def _prep_inputs(nsteps, key, values, lens, text, emb, W_ih1, W_hh1, b_ih1,
                 b_hh1, W_ih2, W_hh2, b_ih2, b_hh2, W_out, b_out):
    """Host-side layout prep. Returns per-core list of input dicts."""
    import ml_dtypes
    bf16 = ml_dtypes.bfloat16
    f = np.float32
    nt, mch = _nt_mch(nsteps)
    W_ih1 = np.asarray(W_ih1, f)
    W_hh1 = np.asarray(W_hh1, f)
    W_ih2 = np.asarray(W_ih2, f)
    W_hh2 = np.asarray(W_hh2, f)
    W_out = np.asarray(W_out, f)
    emb = np.ascontiguousarray(np.asarray(emb, f))

    # z1 moving chunks: [128, 5, 2048]; k-chunk 0 = W_ctx.T, 1..4 = W_hh1.T/2
    W1r = np.empty((128, 5, G1), f)
    W1r[:, 0, :] = W_ih1[:, H:H + VS].T
    for j in range(4):
        W1r[:, 1 + j, :] = 0.5 * W_hh1[:, 128 * j:128 * (j + 1)].T
    # z2 moving chunks: [128, 5, 512]; 0..3 = W_ih2.T/2, 4 = W_hh2.T/2
    W2r = np.empty((128, 5, G2), f)
    for j in range(4):
        W2r[:, j, :] = 0.5 * W_ih2[:, 128 * j:128 * (j + 1)].T
    W2r[:, 4, :] = 0.5 * W_hh2.T
    WoutT = np.empty((128, 2, V), f)
    WoutT[:, 0, :] = 0.5 * W_out[:, :KS].T
    WoutT[:, 1, :] = W_out[:, KS:].T

    # z_emb table: emb @ W_ih1[:, :H].T + b1; row 0 = b1 (padding emb = 0)
    b1 = np.asarray(b_ih1, f) + np.asarray(b_hh1, f)
    emb2 = np.ascontiguousarray(emb @ W_ih1[:, :H].T + b1[None, :])
    b2row = (np.asarray(b_ih2, f) + np.asarray(b_hh2, f)).reshape(1, G2)

    ident = np.eye(128, dtype=f)
    identb_np = np.eye(128, dtype=bf16)
    onesr = np.ones((1, 128), f)
    dmask = np.zeros((128, B * B), f)
    for n in range(B):
        dmask[:, B * n + n] = 1.0

    shared = dict(emb2=emb2, W1r=W1r, W2r=W2r, WoutT=WoutT,
                  b2row=b2row, ident=ident, identb=identb_np, onesr=onesr,
                  dmask=dmask)

    per_core = []
    for c in range(NCORES):
        sl = slice(B * c, B * (c + 1))
        k_c = np.asarray(key[:, sl, :], f)       # (500, 8, 128)
        v_c = np.asarray(values[:, sl, :], f)
        lens_c = np.asarray(lens[sl])
        text_c = np.asarray(text[sl, :])
        keysT = np.ascontiguousarray(0.5 * k_c.transpose(2, 1, 0))
        vals = np.ascontiguousarray(
            v_c.reshape(TCH, TSUB, B, VS).transpose(1, 2, 0, 3)).astype(bf16)
        mask8 = np.where(np.arange(T)[None, :] >= lens_c[:, None], NEG, 0.0)
        mask8 = np.ascontiguousarray(mask8.astype(f))
        # per-step gather indices into emb2: tidxN[n, t] = text[n, t]
        tidxN = np.ascontiguousarray(text_c[:, :nsteps].astype(np.int32))
        ctx0T = np.ascontiguousarray(v_c[0].T)
        d = dict(shared)
        d.update(keysT=keysT, vals=vals, mask8=mask8, tidxN=tidxN,
                 ctx0T=ctx0T)
        per_core.append(d)
    return per_core


def build(ctx: ExitStack, tc, out_ap, ins, nsteps=L):
    import concourse.bass as bass
    from concourse import mybir

    ablate = set(os.environ.get("DEC_ABLATE", "").split(","))

    nc = tc.nc
    f32 = mybir.dt.float32
    f32r = mybir.dt.float32r
    bf16 = mybir.dt.bfloat16
    AF = mybir.ActivationFunctionType
    OP = mybir.AluOpType
    nt, mch = _nt_mch(nsteps)

    mm = nc.tensor.matmul

    consts = ctx.enter_context(tc.tile_pool(name="consts", bufs=1))
    hists = ctx.enter_context(tc.tile_pool(name="hists", bufs=1))

    def load_const(name, dtype=f32):
        a = ins[name]
        t = consts.tile(list(a.shape), dtype, tag=name)
        nc.sync.dma_start(t[:], a[:])
        return t

    W1r = load_const("W1r", f32r)        # [128, 5, 2048]
    W2r = load_const("W2r", f32r)        # [128, 5, 512]
    keysT = load_const("keysT", f32r)    # [128, 8, 500]
    vals = load_const("vals", bf16)      # [125, 8, 4, 128] bf16 stationaries
    mask8 = load_const("mask8", f32r)    # [8, 500]
    b2row = load_const("b2row", f32r)    # [1, 512]
    ident = load_const("ident")    # [128, 128] fp32, for transposes
    identb = load_const("identb", bf16)  # [128, 128] bf16 identity
    onesr = load_const("onesr", f32r)    # [1, 128]
    dmask = load_const("dmask", f32r)    # [128, 64]
    ctx0T = load_const("ctx0T", f32r)    # [128, 8]
    tidxN = load_const("tidxN", mybir.dt.int32)  # [8, nsteps]
    identr = consts.tile([128, 128], f32r, tag="identr")
    nc.gpsimd.dma_start(identr[:], ins["ident"][:])
    emb2_ap = ins["emb2"]                # [V, 2048] f32r, stays in DRAM

    # histories: slot s holds the state after step s-1 (slot 0 = initial)
    H2h = hists.tile([128, B * (nsteps + 1)], f32r)
    CXh = hists.tile([128, B * (nsteps + 1)], f32r)
    H1t = hists.tile([128, 2, HCH * B], f32r)   # ping-pong h1T (2h scale)
    S1 = hists.tile([B, 2, H], f32)             # 2*c1
    S2 = hists.tile([B, 2, KS], f32)            # 2*c2
    nc.gpsimd.memset(H2h[:, 0:B].bitcast(f32), 0.0)
    nc.vector.tensor_copy(CXh[:, 0:B], ctx0T[:])
    nc.gpsimd.memset(H1t[:, 0, :].bitcast(f32), 0.0)
    nc.gpsimd.memset(S1[:, 0, :], 0.0)
    nc.gpsimd.memset(S2[:, 0, :], 0.0)

    I8 = ident[0:B, 0:B]
    I8r = identr[0:B, 0:B]

    # ============ phase B: the recurrence ================================
    with tc.tile_pool(name="zemb", bufs=3) as zemb_p, \
         tc.tile_pool(name="gates", bufs=3) as gates, \
         tc.tile_pool(name="small", bufs=3) as small, \
         tc.tile_pool(name="ps_z1", bufs=1, space="PSUM") as ps_z1, \
         tc.tile_pool(name="ps_z2", bufs=1, space="PSUM") as ps_z2, \
         tc.tile_pool(name="ps_en", bufs=1, space="PSUM") as ps_en, \
         tc.tile_pool(name="ps_sm", bufs=1, space="PSUM") as ps_sm:

        z1q = [ps_z1.tile([B, 512], f32, tag=f"z1q{q}", name=f"z1q{q}")
               for q in range(4)]
        z2_ps = ps_z2.tile([B, G2], f32)
        en_ps = ps_en.tile([B, T], f32)

        def dummy_mm(ncols=256):
            """Const matmul to keep the PE activity monitor busy during
            sparse (non-PE) stretches; results are never read."""
            mm(z2_ps[0:B, 0:ncols], I8r, mask8[:, 0:ncols], start=True,
               stop=True, skip_group_check=True)

        def lstm_gates(zsl, S, pp, w, h_out, keepalive=False):
            """zsl(k) [B, w] PSUM gate chunks -> h_out (= 2h). Order i,f,g,o."""
            ti = gates.tile([B, w], f32, tag=f"ti{w}")
            tf = gates.tile([B, w], f32, tag=f"tf{w}")
            tg = gates.tile([B, w], f32, tag=f"tg{w}")
            to = gates.tile([B, w], f32, tag=f"to{w}")
            nc.scalar.activation(tf[:], zsl(1), AF.Tanh, scale=0.5)
            nc.scalar.activation(tg[:], zsl(2), AF.Tanh)
            nc.scalar.activation(ti[:], zsl(0), AF.Tanh, scale=0.5)
            nc.scalar.activation(to[:], zsl(3), AF.Tanh, scale=0.5)
            fc = gates.tile([B, w], f32, tag=f"fc{w}")
            u = gates.tile([B, w], f32, tag=f"u{w}")
            tcn = gates.tile([B, w], f32, tag=f"tc{w}")
            nc.vector.scalar_tensor_tensor(fc[:], tf[:], 1.0, S[:, pp ^ 1, :],
                                           op0=OP.add, op1=OP.mult)
            if keepalive:
                # HAM keepalive: a tiny PE op gated on a mid-gates value so
                # the PE idle span during the gate chain stays < ~3.4us.
                mm(z2_ps[0:B, 0:B], I8, fc[:, 0:B], start=True, stop=True,
                   skip_group_check=True)
            nc.vector.scalar_tensor_tensor(u[:], ti[:], 1.0, tg[:],
                                           op0=OP.add, op1=OP.mult)
            nc.vector.scalar_tensor_tensor(S[:, pp, :], fc[:], 0.5, u[:],
                                           op0=OP.mult, op1=OP.add)
            nc.scalar.activation(tcn[:], S[:, pp, :], AF.Tanh, scale=0.5)
            nc.vector.scalar_tensor_tensor(h_out[:], to[:], 1.0, tcn[:],
                                           op0=OP.add, op1=OP.mult)

        def gather_zemb(t):
            """Indirect-gather the 8 precomputed z_emb rows for step t."""
            zt = zemb_p.tile([B, G1], f32r, tag="zemb")
            nc.gpsimd.indirect_dma_start(
                out=zt[:], out_offset=None, in_=emb2_ap[:],
                in_offset=bass.IndirectOffsetOnAxis(
                    ap=tidxN[:, t:t + 1], axis=0))
            return zt

        def z1_inject(zt, qs):
            for q in qs:
                sl = slice(512 * q, 512 * (q + 1))
                mm(z1q[q][:], I8r, zt[:, sl], start=True, stop=False,
                   skip_group_check=True)

        def z1_hmms(po, qs):
            for q in qs:
                sl = slice(512 * q, 512 * (q + 1))
                for j in range(4):
                    mm(z1q[q][:], H1t[:, po, B * j:B * (j + 1)],
                       W1r[:, 1 + j, sl], start=False, stop=False,
                       skip_group_check=True)

        zt_next = gather_zemb(0)
        z1_inject(zt_next, (0, 1, 2, 3))
        z1_hmms(0, (0, 1, 2, 3))

        for t in range(nsteps):
            pp = (t + 1) % 2
            po = t % 2
            # -- z1 ctx part closes the accumulation groups; q order
            #    matches the gate ACT issue order (f, g, i, o) ----------
            cxT = CXh[:, B * t:B * (t + 1)]
            for q in (1, 2, 0, 3):
                sl = slice(512 * q, 512 * (q + 1))
                mm(z1q[q][:], cxT, W1r[:, 0, sl],
                   start=False, stop=True, skip_group_check=True)
            if t + 1 < nsteps:
                zt_next = gather_zemb(t + 1)
            # back-to-back filler through the gate-ACT stretch: with the cx
            # matmuls + injects this forms a contiguous busy runway so the
            # activity monitor restores full clock before z2/hmms/energy
            for _ in range(3):
                dummy_mm(500)
            # -- gates 1; h1 rows -> H1t --------------------------------
            h1r = gates.tile([B, H], f32, tag="h1r")
            lstm_gates(lambda k: z1q[k][:], S1, pp, H, h1r, keepalive=True)
            # next step's zemb injects fill the PE during the gate chain
            # (they wait on this step's four gate ACT reads of z1_ps)
            if t + 1 < nsteps:
                z1_inject(zt_next, (0, 1, 2, 3))
            for _ in range(2):
                dummy_mm(500)
            tp = ps_sm.tile([128, 4 * B], f32, tag="tp")
            for j in range(HCH):
                nc.tensor.transpose(tp[:, B * j:B * (j + 1)],
                                    h1r[:, 128 * j:128 * (j + 1)], I8)
            nc.vector.tensor_copy(H1t[:, pp, :], tp[:])
            # -- z2: the two gates1-independent matmuls (bias, W_hh2 @ h2)
            #    go first so the scheduler can hoist them into idle PE time
            mm(z2_ps[:], onesr[:, 0:B], b2row[:], start=True,
               stop=False, skip_group_check=True)
            mm(z2_ps[:], H2h[:, B * t:B * (t + 1)], W2r[:, 4, :],
               start=False, stop=False, skip_group_check=True)
            for j in range(4):
                mm(z2_ps[:], H1t[:, pp, B * j:B * (j + 1)],
                   W2r[:, j, :], start=False, stop=(j == 3),
                   skip_group_check=True)
            # next step's z1 h-part fills the PE during gates2 + softmax
            if t + 1 < nsteps:
                z1_hmms(pp, (0, 1))
            # -- gates 2; h2 rows -> H2h slot t+1 ----------------------
            h2r = gates.tile([B, KS], f32, tag="h2r")
            lstm_gates(lambda k: z2_ps[:, KS * k:KS * (k + 1)], S2, pp, KS,
                       h2r)
            h2T = H2h[:, B * (t + 1):B * (t + 2)]
            tp2 = ps_sm.tile([128, 4 * B], f32, tag="tp")
            nc.tensor.transpose(tp2[:, 0:B], h2r[:], I8)
            nc.vector.tensor_copy(h2T, tp2[:, 0:B])
            if "noattn" in ablate:
                nc.vector.tensor_copy(CXh[:, B * (t + 1):B * (t + 2)],
                                      CXh[:, B * t:B * (t + 1)])
                continue
            # -- energy: dense [8, 500] --------------------------------
            zh2 = small.tile([128, B, B], f32r, tag="zh2")
            nc.vector.tensor_tensor(
                zh2[:],
                h2T.rearrange("p (a n) -> p a n", a=1).to_broadcast(
                    [128, B, B]),
                dmask[:].rearrange("p (a b) -> p a b", a=B), op=OP.mult)
            mm(en_ps[:], I8r, mask8[:], start=True, stop=False,
               skip_group_check=True)
            for n in range(B):
                mm(en_ps[:], zh2[:, n, :], keysT[:, n, :],
                   start=False, stop=(n == B - 1), skip_group_check=True)
            if t + 1 < nsteps:
                z1_hmms(pp, (2, 3))
            # -- softmax (bf16 tail) -----------------------------------
            att = small.tile([B, T], bf16, tag="att")
            atts = small.tile([B, T], bf16, tag="atts")
            den = small.tile([B, 1], f32, tag="den")
            rden = small.tile([B, 1], f32, tag="rden")
            nc.scalar.activation(att[:], en_ps[:], AF.Exp, accum_out=den[:])
            dummy_mm()
            nc.vector.reciprocal(rden[:], den[:])
            nc.vector.tensor_scalar_mul(atts[:], att[:], rden[:, 0:1])
            dummy_mm()
            # -- attn transpose + ctx ----------------------------------
            tp3 = ps_sm.tile([128, 4 * B], bf16, tag="tp")
            for cch in range(TCH):
                nc.tensor.transpose(tp3[0:TSUB, B * cch:B * (cch + 1)],
                                    atts[:, TSUB * cch:TSUB * (cch + 1)],
                                    identb[0:B, 0:B])
            attT = small.tile([128, TCH, B], bf16, tag="attT")
            nc.vector.tensor_copy(
                attT[0:TSUB].rearrange("p a b -> p (a b)"), tp3[0:TSUB, :])
            dummy_mm()
            cx_ps = ps_sm.tile([128, B], f32, tag="cxps")
            for n in range(B):
                for cch in range(TCH):
                    mm(cx_ps[:, n:n + 1], vals[:, n, cch, :],
                       attT[0:TSUB, cch, n:n + 1],
                       start=(n == 0 and cch == 0),
                       stop=(n == B - 1 and cch == TCH - 1),
                       skip_group_check=True)
            nc.vector.tensor_copy(CXh[:, B * (t + 1):B * (t + 2)], cx_ps[:])
            dummy_mm()

    # ============ phase C: logits ========================================
    with tc.tile_pool(name="phc_w", bufs=2) as phc_w, \
         tc.tile_pool(name="phc_o", bufs=3) as phc_o, \
         tc.tile_pool(name="phc_ps", bufs=8, space="PSUM") as phc_ps:
        vchunks = [(512 * q, min(512, V - 512 * q))
                   for q in range((V + 511) // 512)]
        groups = [vchunks[i:i + 4] for i in range(0, len(vchunks), 4)]
        for grp in groups:
            g0 = grp[0][0]
            gw = sum(w for _, w in grp)
            wg = phc_w.tile([128, 2, 2048], f32r, tag="wg")
            nc.sync.dma_start(wg[:, :, 0:gw], ins["WoutT"][:, :, g0:g0 + gw])
            for m in range(mch):
                rows = min(128, nt - 128 * m)
                h2blk = H2h[:, B + 128 * m:B + 128 * m + rows]
                cxblk = CXh[:, B + 128 * m:B + 128 * m + rows]
                ot = phc_o.tile([128, 2048], f32, tag="ot")
                for qi, (q0, qw) in enumerate(grp):
                    nsl = slice(q0 - g0, q0 - g0 + qw)
                    ps = phc_ps.tile([128, 512], f32, tag="lg")
                    mm(ps[0:rows, 0:qw], h2blk, wg[:, 0, nsl],
                       start=True, stop=False)
                    mm(ps[0:rows, 0:qw], cxblk, wg[:, 1, nsl],
                       start=False, stop=True)
                    if qi % 2 == 0:
                        nc.scalar.copy(ot[0:rows, nsl], ps[0:rows, 0:qw])
                    else:
                        nc.vector.tensor_copy(ot[0:rows, nsl],
                                              ps[0:rows, 0:qw])
                nc.sync.dma_start(out_ap[128 * m:128 * m + rows, g0:g0 + gw],
                                  ot[0:rows, 0:gw])


def _build_program(nsteps):
    import concourse.tile as tile
    from concourse import bacc, mybir

    nt, mch = _nt_mch(nsteps)
    nc = bacc.Bacc("TRN2", target_bir_lowering=False, debug=False,
                   num_devices=NCORES)
    shapes = dict(
        emb2=(V, G1), W1r=(128, 5, G1), W2r=(128, 5, G2),
        WoutT=(128, 2, V), b2row=(1, G2),
        ident=(128, 128), identb=(128, 128), onesr=(1, 128),
        dmask=(128, B * B),
        keysT=(128, B, T), vals=(TSUB, B, TCH, VS), mask8=(B, T),
        ctx0T=(128, B),
    )
    F32R_INS = {"emb2", "W1r", "W2r", "WoutT", "b2row",
                "onesr", "dmask", "keysT", "mask8", "ctx0T"}
    BF16_INS = {"vals", "identb"}
    ins = {}
    for name, shp in shapes.items():
        if name in BF16_INS:
            dt_ = mybir.dt.bfloat16
        elif name in F32R_INS:
            dt_ = mybir.dt.float32r
        else:
            dt_ = mybir.dt.float32
        ins[name] = nc.dram_tensor(name, list(shp), dt_,
                                   kind="ExternalInput").ap()
    ins["tidxN"] = nc.dram_tensor("tidxN", [B, nsteps], mybir.dt.int32,
                                  kind="ExternalInput").ap()
    out = nc.dram_tensor("out", [nt, V], mybir.dt.float32,
                         kind="ExternalOutput").ap()
    with ExitStack() as ctx:
        tc = ctx.enter_context(tile.TileContext(nc))
        build(ctx, tc, out, ins, nsteps=nsteps)
    nc.compile()
    return nc


def kernel(**inputs) -> np.ndarray:
    from concourse.bass_utils import run_bass_kernel_spmd

    nsteps = int(os.environ.get("DEC_NSTEPS", L))
    per_core = _prep_inputs(nsteps, **inputs)
    if nsteps not in _cache:
        _cache[nsteps] = _build_program(nsteps)
    nc = _cache[nsteps]
    res = run_bass_kernel_spmd(
        nc, per_core, core_ids=list(range(NCORES)),
        trace=bool(int(os.environ.get("DEC_TRACE", "0"))),
    )
    outs = []
    for c in range(NCORES):
        o = res.results[c]["out"]        # [nt, 8000], rows t*8+n
        outs.append(o.reshape(nsteps, B, V).transpose(1, 0, 2))
    full = np.concatenate(outs, axis=0)  # (64, nsteps, 8000)
    full += np.asarray(inputs["b_out"], np.float32)[None, None, :]
    kernel.last_results = res
    return full



# revision 55
# speedup vs baseline: 1.0011x; 1.0011x over previous
"""Attention-LSTM decoder (LAS-style) Trainium2 Bass kernel.

Sharding: data-parallel over batch N=64 -> 8 cores x 8 examples.

Device strategy (per core, b=8 examples):
 - Recurrent matmuls are batch-stationary (lhsT = state columns [K, 8]) with
   weights streamed as float32r (full-rate fp32 for N>=256).
 - emb(x_t) @ W_ih1[:, :512].T is precomputed for all steps in phase A
   (indirect-DMA gather + big matmul), biases folded in.
 - sigmoid via tanh identity; states stored scaled by 2 (S=2c, H=2h) so each
   gate is one scalar_tensor_tensor; the 2x on h is compensated by
   pre-halving W_hh1/W_ih2/W_hh2/keys/W_out[:, :128] on the host.
 - Attention energy computed DENSE [8, 500] by accumulating 8 matmuls whose
   stationary is H2t masked to one column (block-diag trick); pad mask added
   via an I8 matmul; exp uses accum_out for the softmax denominator (no max
   subtraction -- energies are small, verified against the reference).
 - ctx computed per (example, T-chunk) with stationary val chunks; lands
   transposed [128, 8] = exactly the layout the next-step z1 matmul and the
   final output matmul need.
 - Phase C: [1600, 256] @ [256, 8000] from the stored h2/ctx histories.
"""

import os
from contextlib import ExitStack

import numpy as np

V, H, KS, VS, T, N, L = 8000, 512, 128, 128, 500, 64, 200
NCORES = 8
B = N // NCORES          # 8 examples per core
TCH = 4                  # T chunks for ctx matmuls
TSUB = T // TCH          # 125
HCH = H // 128           # 4 chunks of the h1 dim
G1 = 4 * H               # 2048
G2 = 4 * KS              # 512
NEG = -1e9

_cache = {}


def _nt_mch(nsteps):
    nt = B * nsteps
    return nt, (nt + 127) // 128


def _prep_inputs(nsteps, key, values, lens, text, emb, W_ih1, W_hh1, b_ih1,
                 b_hh1, W_ih2, W_hh2, b_ih2, b_hh2, W_out, b_out):
    """Host-side layout prep. Returns per-core list of input dicts."""
    import ml_dtypes
    bf16 = ml_dtypes.bfloat16
    f = np.float32
    nt, mch = _nt_mch(nsteps)
    W_ih1 = np.asarray(W_ih1, f)
    W_hh1 = np.asarray(W_hh1, f)
    W_ih2 = np.asarray(W_ih2, f)
    W_hh2 = np.asarray(W_hh2, f)
    W_out = np.asarray(W_out, f)
    emb = np.ascontiguousarray(np.asarray(emb, f))

    # z1 moving chunks: [128, 5, 2048]; k-chunk 0 = W_ctx.T, 1..4 = W_hh1.T/2
    W1r = np.empty((128, 5, G1), f)
    W1r[:, 0, :] = W_ih1[:, H:H + VS].T
    for j in range(4):
        W1r[:, 1 + j, :] = 0.5 * W_hh1[:, 128 * j:128 * (j + 1)].T
    # z2 moving chunks: [128, 5, 512]; 0..3 = W_ih2.T/2, 4 = W_hh2.T/2
    W2r = np.empty((128, 5, G2), f)
    for j in range(4):
        W2r[:, j, :] = 0.5 * W_ih2[:, 128 * j:128 * (j + 1)].T
    W2r[:, 4, :] = 0.5 * W_hh2.T
    WoutT = np.empty((128, 2, V), f)
    WoutT[:, 0, :] = 0.5 * W_out[:, :KS].T
    WoutT[:, 1, :] = W_out[:, KS:].T

    # z_emb table: emb @ W_ih1[:, :H].T + b1; row 0 = b1 (padding emb = 0)
    b1 = np.asarray(b_ih1, f) + np.asarray(b_hh1, f)
    emb2 = np.ascontiguousarray(emb @ W_ih1[:, :H].T + b1[None, :])
    b2row = (np.asarray(b_ih2, f) + np.asarray(b_hh2, f)).reshape(1, G2)

    ident = np.eye(128, dtype=f)
    identb_np = np.eye(128, dtype=bf16)
    onesr = np.ones((1, 128), f)
    dmask = np.zeros((128, B * B), f)
    for n in range(B):
        dmask[:, B * n + n] = 1.0

    shared = dict(emb2=emb2, W1r=W1r, W2r=W2r, WoutT=WoutT,
                  b2row=b2row, ident=ident, identb=identb_np, onesr=onesr,
                  dmask=dmask)

    per_core = []
    for c in range(NCORES):
        sl = slice(B * c, B * (c + 1))
        k_c = np.asarray(key[:, sl, :], f)       # (500, 8, 128)
        v_c = np.asarray(values[:, sl, :], f)
        lens_c = np.asarray(lens[sl])
        text_c = np.asarray(text[sl, :])
        keysT = np.ascontiguousarray(0.5 * k_c.transpose(2, 1, 0))
        vals = np.ascontiguousarray(
            v_c.reshape(TCH, TSUB, B, VS).transpose(1, 2, 0, 3)).astype(bf16)
        mask8 = np.where(np.arange(T)[None, :] >= lens_c[:, None], NEG, 0.0)
        mask8 = np.ascontiguousarray(mask8.astype(f))
        # per-step gather indices into emb2: tidxN[n, t] = text[n, t]
        tidxN = np.ascontiguousarray(text_c[:, :nsteps].astype(np.int32))
        ctx0T = np.ascontiguousarray(v_c[0].T)
        d = dict(shared)
        d.update(keysT=keysT, vals=vals, mask8=mask8, tidxN=tidxN,
                 ctx0T=ctx0T)
        per_core.append(d)
    return per_core


def build(ctx: ExitStack, tc, out_ap, ins, nsteps=L):
    import concourse.bass as bass
    from concourse import mybir

    ablate = set(os.environ.get("DEC_ABLATE", "").split(","))

    nc = tc.nc
    f32 = mybir.dt.float32
    f32r = mybir.dt.float32r
    bf16 = mybir.dt.bfloat16
    AF = mybir.ActivationFunctionType
    OP = mybir.AluOpType
    nt, mch = _nt_mch(nsteps)

    mm = nc.tensor.matmul

    consts = ctx.enter_context(tc.tile_pool(name="consts", bufs=1))
    hists = ctx.enter_context(tc.tile_pool(name="hists", bufs=1))

    def load_const(name, dtype=f32):
        a = ins[name]
        t = consts.tile(list(a.shape), dtype, tag=name)
        nc.sync.dma_start(t[:], a[:])
        return t

    W1r = load_const("W1r", f32r)        # [128, 5, 2048]
    W2r = load_const("W2r", f32r)        # [128, 5, 512]
    keysT = load_const("keysT", f32r)    # [128, 8, 500]
    vals = load_const("vals", bf16)      # [125, 8, 4, 128] bf16 stationaries
    mask8 = load_const("mask8", f32r)    # [8, 500]
    b2row = load_const("b2row", f32r)    # [1, 512]
    ident = load_const("ident")    # [128, 128] fp32, for transposes
    identb = load_const("identb", bf16)  # [128, 128] bf16 identity
    onesr = load_const("onesr", f32r)    # [1, 128]
    dmask = load_const("dmask", f32r)    # [128, 64]
    ctx0T = load_const("ctx0T", f32r)    # [128, 8]
    tidxN = load_const("tidxN", mybir.dt.int32)  # [8, nsteps]
    identr = consts.tile([128, 128], f32r, tag="identr")
    nc.gpsimd.dma_start(identr[:], ins["ident"][:])
    emb2_ap = ins["emb2"]                # [V, 2048] f32r, stays in DRAM

    # histories: slot s holds the state after step s-1 (slot 0 = initial)
    H2h = hists.tile([128, B * (nsteps + 1)], f32r)
    CXh = hists.tile([128, B * (nsteps + 1)], f32r)
    H1t = hists.tile([128, 2, HCH * B], f32r)   # ping-pong h1T (2h scale)
    S1 = hists.tile([B, 2, H], f32)             # 2*c1
    S2 = hists.tile([B, 2, KS], f32)            # 2*c2
    nc.gpsimd.memset(H2h[:, 0:B].bitcast(f32), 0.0)
    nc.vector.tensor_copy(CXh[:, 0:B], ctx0T[:])
    nc.gpsimd.memset(H1t[:, 0, :].bitcast(f32), 0.0)
    nc.gpsimd.memset(S1[:, 0, :], 0.0)
    nc.gpsimd.memset(S2[:, 0, :], 0.0)

    I8 = ident[0:B, 0:B]
    I8r = identr[0:B, 0:B]

    # phase-C weight pool at build scope: its SBUF is disjoint from the
    # recurrence pools, so the wg DMAs prefetch during phase B instead of
    # stalling ~15us at the recurrence/phase-C boundary.
    phc_w = ctx.enter_context(tc.tile_pool(name="phc_w", bufs=2))

    # ============ phase B: the recurrence ================================
    with tc.tile_pool(name="zemb", bufs=2) as zemb_p, \
         tc.tile_pool(name="gates", bufs=2) as gates, \
         tc.tile_pool(name="small", bufs=3) as small, \
         tc.tile_pool(name="ps_z1", bufs=1, space="PSUM") as ps_z1, \
         tc.tile_pool(name="ps_z2", bufs=1, space="PSUM") as ps_z2, \
         tc.tile_pool(name="ps_en", bufs=1, space="PSUM") as ps_en, \
         tc.tile_pool(name="ps_sm", bufs=1, space="PSUM") as ps_sm:

        z1q = [ps_z1.tile([B, 512], f32, tag=f"z1q{q}", name=f"z1q{q}")
               for q in range(4)]
        z2_ps = ps_z2.tile([B, G2], f32)
        en_ps = ps_en.tile([B, T], f32)

        def dummy_mm(ncols=256):
            """Const matmul to keep the PE activity monitor busy during
            sparse (non-PE) stretches; results are never read."""
            mm(z2_ps[0:B, 0:ncols], I8r, mask8[:, 0:ncols], start=True,
               stop=True, skip_group_check=True)

        def lstm_gates(zsl, S, pp, w, h_out, keepalive=False):
            """zsl(k) [B, w] PSUM gate chunks -> h_out (= 2h). Order i,f,g,o."""
            ti = gates.tile([B, w], f32, tag=f"ti{w}")
            tf = gates.tile([B, w], f32, tag=f"tf{w}")
            tg = gates.tile([B, w], f32, tag=f"tg{w}")
            to = gates.tile([B, w], f32, tag=f"to{w}")
            nc.scalar.activation(tf[:], zsl(1), AF.Tanh, scale=0.5)
            nc.scalar.activation(tg[:], zsl(2), AF.Tanh)
            nc.scalar.activation(ti[:], zsl(0), AF.Tanh, scale=0.5)
            nc.scalar.activation(to[:], zsl(3), AF.Tanh, scale=0.5)
            fc = gates.tile([B, w], f32, tag=f"fc{w}")
            u = gates.tile([B, w], f32, tag=f"u{w}")
            tcn = gates.tile([B, w], f32, tag=f"tc{w}")
            nc.vector.scalar_tensor_tensor(fc[:], tf[:], 1.0, S[:, pp ^ 1, :],
                                           op0=OP.add, op1=OP.mult)
            if keepalive:
                # HAM keepalive: a tiny PE op gated on a mid-gates value so
                # the PE idle span during the gate chain stays < ~3.4us.
                mm(z2_ps[0:B, 0:B], I8, fc[:, 0:B], start=True, stop=True,
                   skip_group_check=True)
            nc.vector.scalar_tensor_tensor(u[:], ti[:], 1.0, tg[:],
                                           op0=OP.add, op1=OP.mult)
            nc.vector.scalar_tensor_tensor(S[:, pp, :], fc[:], 0.5, u[:],
                                           op0=OP.mult, op1=OP.add)
            nc.scalar.activation(tcn[:], S[:, pp, :], AF.Tanh, scale=0.5)
            nc.vector.scalar_tensor_tensor(h_out[:], to[:], 1.0, tcn[:],
                                           op0=OP.add, op1=OP.mult)

        def gather_zemb(t):
            """Indirect-gather the 8 precomputed z_emb rows for step t."""
            zt = zemb_p.tile([B, G1], f32r, tag="zemb")
            nc.gpsimd.indirect_dma_start(
                out=zt[:], out_offset=None, in_=emb2_ap[:],
                in_offset=bass.IndirectOffsetOnAxis(
                    ap=tidxN[:, t:t + 1], axis=0))
            return zt

        def z1_inject(zt, qs):
            for q in qs:
                sl = slice(512 * q, 512 * (q + 1))
                mm(z1q[q][:], I8r, zt[:, sl], start=True, stop=False,
                   skip_group_check=True)

        def z1_hmms(po, qs):
            for q in qs:
                sl = slice(512 * q, 512 * (q + 1))
                for j in range(4):
                    mm(z1q[q][:], H1t[:, po, B * j:B * (j + 1)],
                       W1r[:, 1 + j, sl], start=False, stop=False,
                       skip_group_check=True)

        zt_next = gather_zemb(0)
        z1_inject(zt_next, (0, 1, 2, 3))
        z1_hmms(0, (0, 1, 2, 3))

        for t in range(nsteps):
            pp = (t + 1) % 2
            po = t % 2
            # -- z1 ctx part closes the accumulation groups; q order
            #    matches the gate ACT issue order (f, g, i, o) ----------
            cxT = CXh[:, B * t:B * (t + 1)]
            for q in (1, 2, 0, 3):
                sl = slice(512 * q, 512 * (q + 1))
                mm(z1q[q][:], cxT, W1r[:, 0, sl],
                   start=False, stop=True, skip_group_check=True)
            if t + 1 < nsteps:
                zt_next = gather_zemb(t + 1)
            # back-to-back filler through the gate-ACT stretch: with the cx
            # matmuls + injects this forms a contiguous busy runway so the
            # activity monitor restores full clock before z2/hmms/energy
            for _ in range(3):
                dummy_mm(500)
            # -- gates 1; h1 rows -> H1t --------------------------------
            h1r = gates.tile([B, H], f32, tag="h1r")
            lstm_gates(lambda k: z1q[k][:], S1, pp, H, h1r, keepalive=True)
            # next step's zemb injects fill the PE during the gate chain
            # (they wait on this step's four gate ACT reads of z1_ps)
            if t + 1 < nsteps:
                z1_inject(zt_next, (0, 1, 2, 3))
            for _ in range(2):
                dummy_mm(500)
            tp = ps_sm.tile([128, 4 * B], f32, tag="tp")
            for j in range(HCH):
                nc.tensor.transpose(tp[:, B * j:B * (j + 1)],
                                    h1r[:, 128 * j:128 * (j + 1)], I8)
            nc.vector.tensor_copy(H1t[:, pp, :], tp[:])
            # -- z2: the two gates1-independent matmuls (bias, W_hh2 @ h2)
            #    go first so the scheduler can hoist them into idle PE time
            mm(z2_ps[:], onesr[:, 0:B], b2row[:], start=True,
               stop=False, skip_group_check=True)
            mm(z2_ps[:], H2h[:, B * t:B * (t + 1)], W2r[:, 4, :],
               start=False, stop=False, skip_group_check=True)
            for j in range(4):
                mm(z2_ps[:], H1t[:, pp, B * j:B * (j + 1)],
                   W2r[:, j, :], start=False, stop=(j == 3),
                   skip_group_check=True)
            # next step's z1 h-part fills the PE during gates2 + softmax
            if t + 1 < nsteps:
                z1_hmms(pp, (0, 1))
            # -- gates 2; h2 rows -> H2h slot t+1 ----------------------
            h2r = gates.tile([B, KS], f32, tag="h2r")
            lstm_gates(lambda k: z2_ps[:, KS * k:KS * (k + 1)], S2, pp, KS,
                       h2r)
            h2T = H2h[:, B * (t + 1):B * (t + 2)]
            tp2 = ps_sm.tile([128, 4 * B], f32, tag="tp")
            nc.tensor.transpose(tp2[:, 0:B], h2r[:], I8)
            nc.vector.tensor_copy(h2T, tp2[:, 0:B])
            if "noattn" in ablate:
                nc.vector.tensor_copy(CXh[:, B * (t + 1):B * (t + 2)],
                                      CXh[:, B * t:B * (t + 1)])
                continue
            # -- energy: dense [8, 500] --------------------------------
            zh2 = small.tile([128, B, B], f32r, tag="zh2")
            nc.vector.tensor_tensor(
                zh2[:],
                h2T.rearrange("p (a n) -> p a n", a=1).to_broadcast(
                    [128, B, B]),
                dmask[:].rearrange("p (a b) -> p a b", a=B), op=OP.mult)
            mm(en_ps[:], I8r, mask8[:], start=True, stop=False,
               skip_group_check=True)
            for n in range(B):
                mm(en_ps[:], zh2[:, n, :], keysT[:, n, :],
                   start=False, stop=(n == B - 1), skip_group_check=True)
            if t + 1 < nsteps:
                z1_hmms(pp, (2, 3))
            # -- softmax (bf16 tail) -----------------------------------
            att = small.tile([B, T], bf16, tag="att")
            atts = small.tile([B, T], bf16, tag="atts")
            den = small.tile([B, 1], f32, tag="den")
            rden = small.tile([B, 1], f32, tag="rden")
            nc.scalar.activation(att[:], en_ps[:], AF.Exp, accum_out=den[:])
            dummy_mm()
            nc.vector.reciprocal(rden[:], den[:])
            nc.vector.tensor_scalar_mul(atts[:], att[:], rden[:, 0:1])
            dummy_mm()
            # -- attn transpose + ctx ----------------------------------
            tp3 = ps_sm.tile([128, 4 * B], bf16, tag="tp")
            for cch in range(TCH):
                nc.tensor.transpose(tp3[0:TSUB, B * cch:B * (cch + 1)],
                                    atts[:, TSUB * cch:TSUB * (cch + 1)],
                                    identb[0:B, 0:B])
            attT = small.tile([128, TCH, B], bf16, tag="attT")
            nc.vector.tensor_copy(
                attT[0:TSUB].rearrange("p a b -> p (a b)"), tp3[0:TSUB, :])
            dummy_mm()
            cx_ps = ps_sm.tile([128, B], f32, tag="cxps")
            for n in range(B):
                for cch in range(TCH):
                    mm(cx_ps[:, n:n + 1], vals[:, n, cch, :],
                       attT[0:TSUB, cch, n:n + 1],
                       start=(n == 0 and cch == 0),
                       stop=(n == B - 1 and cch == TCH - 1),
                       skip_group_check=True)
            nc.vector.tensor_copy(CXh[:, B * (t + 1):B * (t + 2)], cx_ps[:])
            dummy_mm()

    # ============ phase C: logits ========================================
    with tc.tile_pool(name="phc_o", bufs=3) as phc_o, \
         tc.tile_pool(name="phc_ps", bufs=8, space="PSUM") as phc_ps:
        vchunks = [(512 * q, min(512, V - 512 * q))
                   for q in range((V + 511) // 512)]
        groups = [vchunks[i:i + 4] for i in range(0, len(vchunks), 4)]
        for grp in groups:
            g0 = grp[0][0]
            gw = sum(w for _, w in grp)
            wg = phc_w.tile([128, 2, 2048], f32r, tag="wg")
            nc.sync.dma_start(wg[:, :, 0:gw], ins["WoutT"][:, :, g0:g0 + gw])
            for m in range(mch):
                rows = min(128, nt - 128 * m)
                h2blk = H2h[:, B + 128 * m:B + 128 * m + rows]
                cxblk = CXh[:, B + 128 * m:B + 128 * m + rows]
                ot = phc_o.tile([128, 2048], f32, tag="ot")
                for qi, (q0, qw) in enumerate(grp):
                    nsl = slice(q0 - g0, q0 - g0 + qw)
                    ps = phc_ps.tile([128, 512], f32, tag="lg")
                    mm(ps[0:rows, 0:qw], h2blk, wg[:, 0, nsl],
                       start=True, stop=False)
                    mm(ps[0:rows, 0:qw], cxblk, wg[:, 1, nsl],
                       start=False, stop=True)
                    if qi % 2 == 0:
                        nc.scalar.copy(ot[0:rows, nsl], ps[0:rows, 0:qw])
                    else:
                        nc.vector.tensor_copy(ot[0:rows, nsl],
                                              ps[0:rows, 0:qw])
                nc.sync.dma_start(out_ap[128 * m:128 * m + rows, g0:g0 + gw],
                                  ot[0:rows, 0:gw])


def _build_program(nsteps):
    import concourse.tile as tile
    from concourse import bacc, mybir

    nt, mch = _nt_mch(nsteps)
    nc = bacc.Bacc("TRN2", target_bir_lowering=False, debug=False,
                   num_devices=NCORES)
    shapes = dict(
        emb2=(V, G1), W1r=(128, 5, G1), W2r=(128, 5, G2),
        WoutT=(128, 2, V), b2row=(1, G2),
        ident=(128, 128), identb=(128, 128), onesr=(1, 128),
        dmask=(128, B * B),
        keysT=(128, B, T), vals=(TSUB, B, TCH, VS), mask8=(B, T),
        ctx0T=(128, B),
    )
    F32R_INS = {"emb2", "W1r", "W2r", "WoutT", "b2row",
                "onesr", "dmask", "keysT", "mask8", "ctx0T"}
    BF16_INS = {"vals", "identb"}
    ins = {}
    for name, shp in shapes.items():
        if name in BF16_INS:
            dt_ = mybir.dt.bfloat16
        elif name in F32R_INS:
            dt_ = mybir.dt.float32r
        else:
            dt_ = mybir.dt.float32
        ins[name] = nc.dram_tensor(name, list(shp), dt_,
                                   kind="ExternalInput").ap()
    ins["tidxN"] = nc.dram_tensor("tidxN", [B, nsteps], mybir.dt.int32,
                                  kind="ExternalInput").ap()
    out = nc.dram_tensor("out", [nt, V], mybir.dt.float32,
                         kind="ExternalOutput").ap()
    with ExitStack() as ctx:
        tc = ctx.enter_context(tile.TileContext(nc))
        build(ctx, tc, out, ins, nsteps=nsteps)
    nc.compile()
    return nc


def kernel(**inputs) -> np.ndarray:
    from concourse.bass_utils import run_bass_kernel_spmd

    nsteps = int(os.environ.get("DEC_NSTEPS", L))
    per_core = _prep_inputs(nsteps, **inputs)
    if nsteps not in _cache:
        _cache[nsteps] = _build_program(nsteps)
    nc = _cache[nsteps]
    res = run_bass_kernel_spmd(
        nc, per_core, core_ids=list(range(NCORES)),
        trace=bool(int(os.environ.get("DEC_TRACE", "0"))),
    )
    outs = []
    for c in range(NCORES):
        o = res.results[c]["out"]        # [nt, 8000], rows t*8+n
        outs.append(o.reshape(nsteps, B, V).transpose(1, 0, 2))
    full = np.concatenate(outs, axis=0)  # (64, nsteps, 8000)
    full += np.asarray(inputs["b_out"], np.float32)[None, None, :]
    kernel.last_results = res
    return full

